# revision 1
# baseline (speedup 1.0000x reference)
"""Multi-head causal attention (B=4, T=2048, C=1024, H=16, D=64) on 8 trn2 cores.

Sharding: core c owns batch b = c//2 and heads g*8..g*8+7 where g = c%2
(batch-parallel x head-tensor-parallel). Each core computes its 8 heads'
QKV projections, causal attention, and a partial output projection
(columns of Wp belonging to its heads). Host sums the two head-group
partials per batch and adds the bias.

Device layout notes (per core):
  xT  [C=1024, T=2048]  host-pretransposed x slice (contraction dim on partitions)
  wq/wk/wv [C=1024, 512] host layout: W[h,c,d] -> [c, h*64+d] for local heads
  wps [512, 1024]        host layout: Wp[c, j]^T slice (rows j = local head dims)
  tri [128, 128]         upper-triangular (incl diag) 0/1 f32 mask
  o   [C=1024, T=2048]   partial out^T (pre-bias)

All matmuls: out = lhsT.T @ rhs, contraction on partitions.
  QT/KT:  lhsT = W[ck-tile, m-tile]   rhs = xT[ck-tile, t-chunk]    -> [m, t]
  V:      lhsT = xT[ck-tile, s-tile]  rhs = Wv[ck-tile, :]          -> [s, hd]
  scores^T: lhsT = KT_h[d, s-tile]    rhs = QT_h[d, t-chunk]        -> [s, t]
  exp on ACT (scale=1/8 fused); no max-subtraction (inputs are scale-0.02
  randn, scores*0.125 stay within ~[-3, 3], exp is safe in f32)
  AV^T:   lhsT = [V_h | 1][s-tile, 65] rhs = expT strip [s-tile, t]  -> [d+sum, t]
  out^T:  lhsT = WpS[j-tile, c-tile]  rhs = YT[j-tile, t-chunk]     -> [c, t]

Unnormalized AV^T rows + the rowsum row accumulate in PSUM; each t-chunk is
normalized (x 1/rowsum broadcast via a rank-1 PE outer product) as soon as
its last strip lands, then staged to a DRAM YT buffer that the projection
phase reads back.
"""

import numpy as np
from contextlib import ExitStack

B, T, C, H, D = 4, 2048, 1024, 16, 64
HL = H // 2          # 8 heads per core
N_CORES = 8
P = 128
NK = C // P          # 8 contraction tiles for projections
NM = HL * D // P     # 4 m-tiles of Q/K head-dims
NS = T // P          # 16 s-tiles (key strips)
CH = 512             # t-chunk width
NCH = T // CH        # 4 t-chunks

_nc_cache = None


def build_nc():
    global _nc_cache
    if _nc_cache is not None:
        return _nc_cache
    import concourse.bass as bass  # noqa: F401
    import concourse.tile as tile
    from concourse import bacc, mybir

    f32 = mybir.dt.float32
    f32r = mybir.dt.float32r
    Exp = mybir.ActivationFunctionType.Exp

    def mm(out, lhsT, rhs, **kw):
        # float32r runs the PE at 1 cycle/row (vs 4 for plain fp32) when the
        # moving dim is >=256; numerics are the PE's relaxed-fp32 path.
        nc.tensor.matmul(out, lhsT=lhsT.bitcast(f32r), rhs=rhs.bitcast(f32r), **kw)

    nc = bacc.Bacc("TRN2", target_bir_lowering=False, debug=False,
                   enable_asserts=True, num_devices=N_CORES)
    xT = nc.dram_tensor("xT", (C, T), f32r, kind="ExternalInput").ap()
    wq = nc.dram_tensor("wq", (C, HL * D), f32r, kind="ExternalInput").ap()
    wk = nc.dram_tensor("wk", (C, HL * D), f32r, kind="ExternalInput").ap()
    wv = nc.dram_tensor("wv", (C, HL * D), f32r, kind="ExternalInput").ap()
    wps = nc.dram_tensor("wps", (HL * D, C), f32r, kind="ExternalInput").ap()
    tri = nc.dram_tensor("tri", (P, 2 * P), f32r, kind="ExternalInput").ap()
    ones = nc.dram_tensor("ones", (P, 2 * P), f32r, kind="ExternalInput").ap()
    o = nc.dram_tensor("o", (C, T), f32, kind="ExternalOutput").ap()
    wqkv = [wq, wk, wv]

    with tile.TileContext(nc) as tc:
        with ExitStack() as ctx:
            ctx.enter_context(nc.allow_low_precision(
                reason="float32r tiles feed the PE fast path; same width as f32"))
            # PSUM: mm pool 3x[128,1024] = 6 banks, av pool 2x[65,512] = 2 banks
            mm_ps = ctx.enter_context(tc.tile_pool(name="mm_ps", bufs=2, space="PSUM"))
            av_ps = ctx.enter_context(tc.tile_pool(name="av_ps", bufs=3, space="PSUM"))
            rps_ps = ctx.enter_context(tc.tile_pool(name="rps_ps", bufs=1, space="PSUM"))

            const_pool = ctx.enter_context(tc.tile_pool(name="const", bufs=1))
            # tri: [128, 256]; left half zeros, right half upper-triangular.
            # Diagonal strips use the right 128 cols; i%4==3 strips use all 256
            # (the zero half clears pool garbage so padded-to-256 AV matmuls
            # read zeros left of the diagonal block).
            tri_sb = const_pool.tile([P, 2 * P], f32r, name="tri_sb", tag="tri_sb")
            nc.sync.dma_start(out=tri_sb, in_=tri)
            ones_sb = const_pool.tile([P, D], f32r, name="ones_sb", tag="ones_sb")
            nc.sync.dma_start(out=ones_sb, in_=ones[:, 0:D])

            # unnormalized-head-output staging lives in DRAM so QKV can use SBUF
            ydram = ctx.enter_context(tc.tile_pool(name="ydram", bufs=1, space="DRAM"))
            ytd = ydram.tile([HL * D, T], f32r, name="ytd", tag="ytd")

            with ExitStack() as qkv_ctx:
                qkpool = qkv_ctx.enter_context(tc.tile_pool(name="qkpool", bufs=1))
                QT = [qkpool.tile([P, T], f32r, name=f"qt{m}", tag=f"qt{m}")
                      for m in range(NM)]
                KT = [qkpool.tile([P, T], f32r, name=f"kt{m}", tag=f"kt{m}")
                      for m in range(NM)]
                # V: [s-within-tile, s-tile, head, d+1]; col 64 = ones (rowsum trick)
                Vsb = qkpool.tile([P, NS, HL, D + 1], f32r, name="vsb", tag="vsb")
                nc.sync.dma_start(
                    out=Vsb[:, :, :, D],
                    in_=ones[:, 0:NS * HL].rearrange("p (s h) -> p s h", s=NS))

                # ---- Phase 1: QKV projections ----
                with ExitStack() as p1:
                    xpool = p1.enter_context(tc.tile_pool(name="xpool", bufs=2))
                    wpool = p1.enter_context(tc.tile_pool(name="wpool", bufs=1))
                    W_sb = []
                    for proj in range(3):
                        row = [wpool.tile([P, HL * D], f32r,
                                          name=f"w{proj}_{k}", tag=f"w{proj}_{k}")
                               for k in range(NK)]
                        for k in range(NK):
                            nc.sync.dma_start(
                                out=row[k], in_=wqkv[proj][k * P:(k + 1) * P, :])
                        W_sb.append(row)
                    for ch in range(NCH):
                        xs = [xpool.tile([P, CH], f32r, name=f"xs{k}", tag=f"xs{k}")
                              for k in range(NK)]
                        for k in range(NK):
                            nc.scalar.dma_start(
                                out=xs[k], in_=xT[k * P:(k + 1) * P, ch * CH:(ch + 1) * CH])
                        # Q and K projections: W stationary, xT moving
                        for proj in range(2):
                            dst = QT if proj == 0 else KT
                            for m in range(NM):
                                ps = mm_ps.tile([P, CH], f32, name="qk_ps", tag="mm")
                                for k in range(NK):
                                    mm(ps, W_sb[proj][k][:, m * P:(m + 1) * P], xs[k],
                                       start=(k == 0), stop=(k == NK - 1))
                                nc.vector.tensor_copy(
                                    dst[m][:, ch * CH:(ch + 1) * CH], ps)
                        # V projection: xT stationary, Wv moving -> [s, h*d]
                        for sl in range(CH // P):
                            s = ch * (CH // P) + sl
                            ps = mm_ps.tile([P, HL * D], f32, name="v_ps", tag="mm")
                            for k in range(NK):
                                mm(ps, xs[k][:, sl * P:(sl + 1) * P], W_sb[2][k],
                                   start=(k == 0), stop=(k == NK - 1))
                            nc.vector.tensor_copy(
                                Vsb[:, s, :, 0:D],
                                ps.rearrange("p (h d) -> p h d", h=HL))

                # ---- Phase 2: attention, globally software-pipelined ----
                # 16 (head, t-half) passes; each strip-pass is one <=1024-wide
                # psum segment + one exp. All passes flatten into ONE pipeline
                # with AV trailing scores/exp by LAG strips, so ACT never idles
                # at pass boundaries (PE issues the next pass's scores before
                # this pass's AV tail).
                with ExitStack() as p2:
                    strip_pool = p2.enter_context(tc.tile_pool(name="strip_pool", bufs=8))
                    small = p2.enter_context(tc.tile_pool(name="small", bufs=3))
                    tmp_pool = p2.enter_context(tc.tile_pool(name="tmp_pool", bufs=3))

                    def make_pass(h, half):
                        mt, off = h // 2, D * (h % 2)
                        tlo = half * 1024
                        ns = 8 if half == 0 else NS
                        st = {"strips": [None] * ns, "avs": None, "tmp": None}

                        def do_scores(i):
                            t0 = P * i
                            s0 = max(t0, tlo)          # first valid col
                            strip = strip_pool.tile([P, 1024], f32r,
                                                    name="strip", tag="strip")
                            st["strips"][i] = strip
                            seg_base = CH * (s0 // CH)
                            ps = mm_ps.tile([P, 1024], f32, name="sc_ps", tag="mm")
                            b0 = s0
                            while b0 < tlo + 1024:
                                b1 = min((b0 // CH + 1) * CH, tlo + 1024)
                                c0 = b0
                                if b1 - b0 == P:
                                    c0 = b0 - P        # pad N=128 -> 256 (f32r)
                                mm(ps[:, c0 - seg_base:b1 - seg_base],
                                   KT[mt][off:off + D, t0:t0 + P],
                                   QT[mt][off:off + D, c0:b1],
                                   start=True, stop=True)
                                b0 = b1
                            # padded first block: exp the pad cols too (junk
                            # the widened mask zeroes; keeps reads initialized)
                            pad = P if s0 % CH == CH - P else 0
                            nc.scalar.activation(
                                strip[:, s0 - pad - tlo:1024],
                                ps[:, s0 - pad - seg_base:tlo + 1024 - seg_base],
                                Exp, scale=float(1.0 / np.sqrt(D)))

                        def do_av(i):
                            if st["avs"] is None:
                                st["avs"] = {j: av_ps.tile([D + 1, CH], f32,
                                                           name=f"av{j}", tag="av")
                                             for j in (2 * half, 2 * half + 1)}
                            avs = st["avs"]
                            t0 = P * i
                            strip = st["strips"][i]
                            if t0 >= tlo:              # diagonal block here
                                if i % 4 == 3:
                                    # widened mask zeroes pool garbage so the
                                    # padded AV matmul reads zeros
                                    nc.vector.tensor_mul(
                                        strip[:, t0 - P - tlo:t0 + P - tlo],
                                        strip[:, t0 - P - tlo:t0 + P - tlo],
                                        tri_sb)
                                else:
                                    nc.vector.tensor_mul(
                                        strip[:, t0 - tlo:t0 + P - tlo],
                                        strip[:, t0 - tlo:t0 + P - tlo],
                                        tri_sb[:, P:2 * P])
                            for j in (2 * half, 2 * half + 1):
                                if CH * (j + 1) <= t0:
                                    continue
                                ts0 = max(CH * j, t0)
                                if CH * (j + 1) - ts0 == P:
                                    ts0 -= P           # padded; mask zeroed cols
                                mm(avs[j][:, ts0 - CH * j:CH],
                                   Vsb[:, i, h, :],
                                   strip[:, ts0 - tlo:CH * (j + 1) - tlo],
                                   start=(i == 0), stop=(i == 4 * j + 3),
                                   skip_group_check=True)
                            # chunk j completes at strip 4j+3: normalize
                            if i % 4 == 3 and i // 4 in avs:
                                j = i // 4
                                if st["tmp"] is None:
                                    st["tmp"] = tmp_pool.tile([D, 1024], f32r,
                                                              name="tmp", tag="tmp")
                                rec = small.tile([D + 1, CH], f32r,
                                                 name="rec", tag="rec")
                                nc.vector.reciprocal(rec[D:D + 1, :],
                                                     avs[j][D:D + 1, :])
                                rps = rps_ps.tile([D, CH], f32, name="rps", tag="rps")
                                mm(rps, ones_sb[D:D + 1, 0:D], rec[D:D + 1, :],
                                   start=True, stop=True)
                                # DVE reads only one PSUM operand; stage the
                                # broadcast reciprocal through SBUF
                                rsb = small.tile([D, CH], f32, name="rsb", tag="rsb")
                                nc.vector.tensor_copy(rsb, rps)
                                nc.vector.tensor_mul(
                                    st["tmp"][:, CH * (j % 2):CH * (j % 2 + 1)],
                                    avs[j][0:D, :], rsb)
                            if i == ns - 1:
                                nc.sync.dma_start(
                                    out=ytd[h * D:(h + 1) * D, tlo:tlo + 1024],
                                    in_=st["tmp"])

                        return ([lambda i=i: do_scores(i) for i in range(ns)],
                                [lambda i=i: do_av(i) for i in range(ns)])

                    sflat, aflat = [], []
                    for h in range(HL):
                        for half in range(2):
                            sc, ac = make_pass(h, half)
                            sflat += sc
                            aflat += ac
                    LAG = 6
                    for idx in range(len(sflat) + LAG):
                        if idx < len(sflat):
                            sflat[idx]()
                        if idx >= LAG:
                            aflat[idx - LAG]()

            # ---- Phase 3: output projection (partial; host adds bias+reduce) ----
            with ExitStack() as p3:
                wppool = p3.enter_context(tc.tile_pool(name="wppool", bufs=1))
                ytpool = p3.enter_context(tc.tile_pool(name="ytpool", bufs=1))
                obpool = p3.enter_context(tc.tile_pool(name="obpool", bufs=3))
                Wp_sb = [wppool.tile([P, C], f32r, name=f"wp{j}", tag=f"wp{j}")
                         for j in range(NM)]
                for j in range(NM):
                    nc.sync.dma_start(out=Wp_sb[j], in_=wps[j * P:(j + 1) * P, :])
                yt_sb = [[ytpool.tile([P, CH], f32r, name=f"yt{j}_{ch}", tag=f"yt{j}_{ch}")
                          for ch in range(NCH)] for j in range(NM)]
                for j in range(NM):
                    for ch in range(NCH):
                        nc.scalar.dma_start(
                            out=yt_sb[j][ch],
                            in_=ytd[j * P:(j + 1) * P, ch * CH:(ch + 1) * CH])
                for ct in range(C // P):
                    ob = obpool.tile([P, T], f32, name="ob", tag="ob")
                    for ch in range(NCH):
                        ps = mm_ps.tile([P, CH], f32, name="p_ps", tag="mm")
                        for j in range(NM):
                            mm(ps, Wp_sb[j][:, ct * P:(ct + 1) * P], yt_sb[j][ch],
                               start=(j == 0), stop=(j == NM - 1))
                        # alternate evacuation between DVE and the otherwise
                        # idle ACT engine to halve the copy chain in the tail
                        if ch % 2 == 0:
                            nc.vector.tensor_copy(ob[:, ch * CH:(ch + 1) * CH], ps)
                        else:
                            nc.scalar.copy(ob[:, ch * CH:(ch + 1) * CH], ps)
                    nc.sync.dma_start(out=o[ct * P:(ct + 1) * P, :], in_=ob)

    nc.compile()
    _nc_cache = nc
    return nc


def make_in_maps(x, Wq, Wk, Wv, Wp):
    """Shard FULL inputs into per-core input maps."""
    tri = np.concatenate(
        [np.zeros((P, P), dtype=np.float32),
         np.triu(np.ones((P, P), dtype=np.float32))], axis=1)
    in_maps = []
    for c in range(N_CORES):
        b, g = c // 2, c % 2
        hs = slice(g * HL, (g + 1) * HL)
        m = {
            "xT": np.ascontiguousarray(x[b].T),
            "wq": np.ascontiguousarray(Wq[hs].transpose(1, 0, 2).reshape(C, HL * D)),
            "wk": np.ascontiguousarray(Wk[hs].transpose(1, 0, 2).reshape(C, HL * D)),
            "wv": np.ascontiguousarray(Wv[hs].transpose(1, 0, 2).reshape(C, HL * D)),
            "wps": np.ascontiguousarray(Wp[:, g * HL * D:(g + 1) * HL * D].T),
            "tri": tri,
            "ones": np.ones((P, 2 * P), dtype=np.float32),
        }
        in_maps.append(m)
    return in_maps


def assemble(results, bp):
    """Sum head-group partials per batch, add bias, transpose back."""
    out = np.empty((B, T, C), dtype=np.float32)
    for b in range(B):
        acc = results[2 * b]["o"] + results[2 * b + 1]["o"]  # [C, T]
        out[b] = acc.T + bp[None, :]
    return out


def kernel(x, Wq, Wk, Wv, Wp, bp):
    from concourse import bass_utils
    x = np.asarray(x, dtype=np.float32)
    nc = build_nc()
    in_maps = make_in_maps(np.asarray(x), np.asarray(Wq), np.asarray(Wk),
                           np.asarray(Wv), np.asarray(Wp))
    res = bass_utils.run_bass_kernel_spmd(nc, in_maps, core_ids=list(range(N_CORES)))
    return assemble(res.results, np.asarray(bp))



# revision 7
# speedup vs baseline: 1.2186x; 1.2186x over previous
"""Multi-head causal attention (B=4, T=2048, C=1024, H=16, D=64) on 8 trn2 cores.

Sharding: core c owns batch b = c//2 and heads g*8..g*8+7 where g = c%2
(batch-parallel x head-tensor-parallel). Each core computes its 8 heads'
QKV projections, causal attention, and a partial output projection
(columns of Wp belonging to its heads). Host sums the two head-group
partials per batch and adds the bias.

All device matmuls are bf16 (f32 PSUM accumulate); rel-err budget is
2e-2 so bf16's ~0.4% is plenty, and bf16 needs none of the f32r
pad-to-256 hacks.

Structure: ONE fused pipeline over 4 head-PAIRS. While pair p's
attention runs (ACT-heavy: exp softmax), pair p+1's QKV projection
matmuls fill the PE, and the output projection rides the tail of pair
3. Scores for the two heads of a pair are emitted interleaved with
K=64 row-tiling (head A in PE rows 0-63, head B in rows 64-127, via
base_partition-derived tile_position) so the two matmuls run
concurrently - ~2x score throughput vs serial heads.

Per-core SBUF layouts:
  xs   8x [128, 2048] bf16   x^T k-tiles, resident
  QT/KT   [128, 2048] bf16   pair's head dims on partitions (A=0:63, B=64:127)
  Vsb     [128, 16, 2, 65]   V strips per s-tile/head, col 64 = ones (rowsum)
  strip   [128, 1024] bf16   exp(scores^T) for one (s-tile, head, t-half)
  Y    4x [128, 2048] bf16   normalized head outputs, j-major (proj rhs)

Attention per (pair, t-half): for each s-strip: scores-pair matmuls
into a [128,1024] PSUM -> exp (ACT, scale=1/8 fused; no max-subtraction,
scores stay in ~[-3,3]) -> diag tri-mask (DVE). AV runs chunk-major
(one live [65,512] PSUM accumulator per head; strips persist in SBUF)
with the ones column giving the rowsum for free; each chunk is
normalized (reciprocal + rank-1 PE broadcast of 1/rowsum) as soon as
its last strip lands, straight into the SBUF-resident Y.

PSUM budget: scores 2x[128,1024]=4 banks, AV 2x[65,512]=2, mm pool
(QKV/proj/rank-1) 2x[128,512]=2 -> 8 banks exactly.
"""

import numpy as np
from contextlib import ExitStack

B, T, C, H, D = 4, 2048, 1024, 16, 64
HL = H // 2          # 8 heads per core
NP = HL // 2         # 4 head-pairs per core
N_CORES = 8
P = 128
NK = C // P          # 8 contraction tiles for projections
NS = T // P          # 16 s-tiles (key strips)
CH = 512             # t-chunk width (PSUM bank)
NCH = T // CH        # 4 t-chunks

_nc_cache = None


def build_nc():
    global _nc_cache
    if _nc_cache is not None:
        return _nc_cache
    import concourse.bass as bass  # noqa: F401
    import concourse.tile as tile
    from concourse import bacc, mybir

    f32 = mybir.dt.float32
    f32r = mybir.dt.float32r
    bf16 = mybir.dt.bfloat16
    Exp = mybir.ActivationFunctionType.Exp

    nc = bacc.Bacc("TRN2", target_bir_lowering=False, debug=False,
                   enable_asserts=True, num_devices=N_CORES)
    xT = nc.dram_tensor("xT", (C, T), bf16, kind="ExternalInput").ap()
    wq = nc.dram_tensor("wq", (C, HL * D), bf16, kind="ExternalInput").ap()
    wk = nc.dram_tensor("wk", (C, HL * D), bf16, kind="ExternalInput").ap()
    wv = nc.dram_tensor("wv", (C, HL * D), bf16, kind="ExternalInput").ap()
    wps = nc.dram_tensor("wps", (HL * D, C), bf16, kind="ExternalInput").ap()
    tri = nc.dram_tensor("tri", (P, P), bf16, kind="ExternalInput").ap()
    onesb = nc.dram_tensor("onesb", (P, NS * 2), bf16, kind="ExternalInput").ap()
    onesf = nc.dram_tensor("onesf", (P, D), f32, kind="ExternalInput").ap()
    o = nc.dram_tensor("o", (C, T), f32, kind="ExternalOutput").ap()
    wqkv = [wq, wk, wv]

    with tile.TileContext(nc) as tc:
        with ExitStack() as ctx:
            ctx.enter_context(nc.allow_low_precision(
                reason="bf16 matmuls/strips; rel-err budget 2e-2"))
            sc_ps = ctx.enter_context(tc.tile_pool(name="sc_ps", bufs=2, space="PSUM"))
            av_ps = ctx.enter_context(tc.tile_pool(name="av_ps", bufs=2, space="PSUM"))
            mm_ps = ctx.enter_context(tc.tile_pool(name="mm_ps", bufs=2, space="PSUM"))

            const_pool = ctx.enter_context(tc.tile_pool(name="const", bufs=1))
            tri_sb = const_pool.tile([P, P], bf16, name="tri_sb", tag="tri_sb")
            nc.sync.dma_start(out=tri_sb, in_=tri)
            ones_sb = const_pool.tile([P, D], f32r, name="ones_sb", tag="ones_sb")
            nc.sync.dma_start(out=ones_sb, in_=onesf.bitcast(f32r))

            # resident inputs
            xpool = ctx.enter_context(tc.tile_pool(name="xpool", bufs=1))
            xs = [xpool.tile([P, T], bf16, name=f"xs{k}", tag=f"xs{k}")
                  for k in range(NK)]
            wpool = ctx.enter_context(tc.tile_pool(name="wpool", bufs=1))
            W_sb = []
            for proj in range(3):
                row = [wpool.tile([P, HL * D], bf16,
                                  name=f"w{proj}_{k}", tag=f"w{proj}_{k}")
                       for k in range(NK)]
                for k in range(NK):
                    nc.sync.dma_start(
                        out=row[k], in_=wqkv[proj][k * P:(k + 1) * P, :])
                W_sb.append(row)
            Wp_sb = [wpool.tile([P, C], bf16, name=f"wp{j}", tag=f"wp{j}")
                     for j in range(NP)]
            for j in range(NP):
                nc.sync.dma_start(out=Wp_sb[j], in_=wps[j * P:(j + 1) * P, :])
            # x chunks last (QKV chain 0 needs all 8 k-tiles of chunk 0)
            for ch in range(NCH):
                for k in range(NK):
                    nc.scalar.dma_start(
                        out=xs[k][:, ch * CH:(ch + 1) * CH],
                        in_=xT[k * P:(k + 1) * P, ch * CH:(ch + 1) * CH])

            # attention outputs (proj rhs), j-major: Y[p] rows = pair p dims
            ypool = ctx.enter_context(tc.tile_pool(name="ypool", bufs=1))
            Y = [ypool.tile([P, T], bf16, name=f"y{j}", tag=f"y{j}")
                 for j in range(NP)]
            obpool = ctx.enter_context(tc.tile_pool(name="obpool", bufs=3))

            qkpool = ctx.enter_context(tc.tile_pool(name="qkpool", bufs=2))
            strip_pool = ctx.enter_context(tc.tile_pool(name="strip_pool", bufs=36))
            small = ctx.enter_context(tc.tile_pool(name="small", bufs=4))

            def mm(out, lhsT, rhs, **kw):
                nc.tensor.matmul(out, lhsT=lhsT, rhs=rhs,
                                 skip_group_check=True, **kw)

            # ---------- QKV for one pair ----------
            # Emission is deferred: hand back a list of thunks (chains) so
            # the caller can interleave them with the prior pair's attention.
            def make_qkv(p):
                QT = qkpool.tile([P, T], bf16, name=f"qt{p}", tag="qt")
                KT = qkpool.tile([P, T], bf16, name=f"kt{p}", tag="kt")
                Vsb = qkpool.tile([P, NS, 2, D + 1], bf16, name=f"v{p}", tag="v")
                chains = []

                def init_ones():
                    nc.sync.dma_start(
                        out=Vsb[:, :, :, D],
                        in_=onesb.rearrange("p (s h) -> p s h", s=NS))
                chains.append(init_ones)

                def qk_chain(proj, ch, dst):
                    ps = mm_ps.tile([P, CH], f32, name="qk_ps", tag="mm")
                    for k in range(NK):
                        mm(ps, W_sb[proj][k][:, p * P:(p + 1) * P],
                           xs[k][:, ch * CH:(ch + 1) * CH],
                           start=(k == 0), stop=(k == NK - 1))
                    nc.vector.tensor_copy(dst[:, ch * CH:(ch + 1) * CH], ps)

                def v_chain(s):
                    ps = mm_ps.tile([P, P], f32, name="v_ps", tag="mm")
                    for k in range(NK):
                        mm(ps, xs[k][:, s * P:(s + 1) * P],
                           W_sb[2][k][:, p * P:(p + 1) * P],
                           start=(k == 0), stop=(k == NK - 1))
                    nc.vector.tensor_copy(
                        Vsb[:, s, :, 0:D],
                        ps.rearrange("p (h d) -> p h d", h=2))

                for ch in range(NCH):
                    chains.append(lambda ch=ch: qk_chain(0, ch, QT))
                    chains.append(lambda ch=ch: qk_chain(1, ch, KT))
                    for sl in range(CH // P):
                        chains.append(lambda s=ch * (CH // P) + sl: v_chain(s))
                return QT, KT, Vsb, chains

            # ---------- attention for one (pair, half) ----------
            def emit_att(p, half, QT, KT, Vsb, filler):
                tlo = half * 1024
                ns = 8 if half == 0 else NS
                strips = [[None, None] for _ in range(ns)]
                fill_iter = iter(filler)

                def fill():
                    u = next(fill_iter, None)
                    if u is not None:
                        u()

                def do_scores(i, g):
                    # head g of the pair: PE rows 64g..64g+63 (row-tiled;
                    # the pair's two streams run concurrently on the array)
                    t0 = P * i
                    s0 = max(t0, tlo)
                    off = D * g
                    strip = strip_pool.tile([P, 1024], bf16,
                                            name="strip", tag="strip")
                    strips[i][g] = strip
                    ps = sc_ps.tile([P, 1024], f32, name="sc_ps", tag="sc")
                    b0 = s0
                    while b0 < tlo + 1024:
                        b1 = min((b0 // CH + 1) * CH, tlo + 1024)
                        mm(ps[:, b0 - tlo:b1 - tlo],
                           KT[off:off + D, t0:t0 + P],
                           QT[off:off + D, b0:b1],
                           start=True, stop=True)
                        b0 = b1
                    nc.scalar.activation(
                        strip[:, s0 - tlo:1024],
                        ps[:, s0 - tlo:1024],
                        Exp, scale=float(1.0 / np.sqrt(D)))
                    if t0 >= tlo:  # mask the diagonal block
                        nc.vector.tensor_mul(
                            strip[:, t0 - tlo:t0 + P - tlo],
                            strip[:, t0 - tlo:t0 + P - tlo],
                            tri_sb)

                def make_av_chunk(j, g):
                    # AV accumulator for t-chunk j, head g (col 64 = rowsum)
                    av = av_ps.tile([D + 1, CH], f32, name="av", tag="av")

                    def av_mm(i):
                        ts0 = max(CH * j, P * i)
                        mm(av[:, ts0 - CH * j:CH],
                           Vsb[:, i, g, :],
                           strips[i][g][:, ts0 - tlo:CH * (j + 1) - tlo],
                           start=(i == 0), stop=(i == 4 * j + 3))

                    def norm():
                        # reciprocal of the rowsum row (partition 64), then a
                        # rank-1 PE outer product broadcasts it over the D rows
                        rec = small.tile([D + 1, CH], f32r, name="rec", tag="rec")
                        nc.vector.reciprocal(rec[D:D + 1, :], av[D:D + 1, :])
                        rps = mm_ps.tile([D, CH], f32, name="rps", tag="mm")
                        nc.tensor.matmul(rps, lhsT=ones_sb[D:D + 1, 0:D],
                                         rhs=rec[D:D + 1, :],
                                         start=True, stop=True,
                                         skip_group_check=True)
                        rsb = small.tile([D, CH], f32, name="rsb", tag="rsb")
                        nc.vector.tensor_copy(rsb, rps)
                        nc.vector.tensor_mul(
                            Y[p][D * g:D * (g + 1), CH * j:CH * (j + 1)],
                            av[0:D, :], rsb)
                    return av_mm, norm

                jlo, jhi = 2 * half, 2 * half + 1
                nlo = 4 * jlo + 4          # strips feeding chunk jlo
                avlo = [make_av_chunk(jlo, g) for g in range(2)]
                avhi = [make_av_chunk(jhi, g) for g in range(2)]

                for i in range(ns):
                    do_scores(i, 0)
                    do_scores(i, 1)
                    fill()
                    il = i - 2             # avlo lags scores for pipelining
                    if 0 <= il < nlo:
                        avlo[0][0](il)
                        avlo[1][0](il)
                        if il == nlo - 1:
                            avlo[0][1]()
                            avlo[1][1]()
                            fill()
                for il in range(max(0, ns - 2), nlo):   # avlo tail
                    avlo[0][0](il)
                    avlo[1][0](il)
                    if il == nlo - 1:
                        avlo[0][1]()
                        avlo[1][1]()
                for i in range(ns):        # chunk jhi: all strips available
                    avhi[0][0](i)
                    avhi[1][0](i)
                    if i % 4 == 3:
                        fill()
                avhi[0][1]()
                avhi[1][1]()
                for u in fill_iter:        # drain remaining filler
                    u()

            # ---------- output projection (per-(ct,ch) flush) ----------
            def proj_unit(ct, ch):
                ps = mm_ps.tile([P, CH], f32, name="p_ps", tag="mm")
                for j in range(NP):
                    mm(ps, Wp_sb[j][:, ct * P:(ct + 1) * P],
                       Y[j][:, ch * CH:(ch + 1) * CH],
                       start=(j == 0), stop=(j == NP - 1))
                ob = obpool.tile([P, CH], f32, name="ob", tag="ob")
                if ct % 2 == 0:
                    nc.vector.tensor_copy(ob, ps)
                else:
                    nc.scalar.copy(ob, ps)
                nc.sync.dma_start(
                    out=o[ct * P:(ct + 1) * P, ch * CH:(ch + 1) * CH], in_=ob)

            # ---------- fused pipeline over pairs ----------
            qkv = make_qkv(0)
            for u in qkv[3]:
                u()
            for p in range(NP):
                if p < NP - 1:
                    nxt = make_qkv(p + 1)
                    filler = list(nxt[3])
                    fh = len(filler) // 3
                    f0, f1 = filler[:fh], filler[fh:]
                else:
                    nxt = None
                    # proj chunks 0/1 need pair-3 half-0 norms (emitted in
                    # half 0), so they ride half 1; chunks 2/3 are the tail.
                    f0 = []
                    f1 = [lambda ct=ct, ch=ch: proj_unit(ct, ch)
                          for ch in (0, 1) for ct in range(C // P)]
                emit_att(p, 0, qkv[0], qkv[1], qkv[2], f0)
                emit_att(p, 1, qkv[0], qkv[1], qkv[2], f1)
                if nxt is not None:
                    qkv = nxt
            for ch in (2, 3):
                for ct in range(C // P):
                    proj_unit(ct, ch)

    nc.compile()
    _nc_cache = nc
    return nc


def make_in_maps(x, Wq, Wk, Wv, Wp):
    """Shard FULL inputs into per-core input maps (bf16 device layouts)."""
    import ml_dtypes
    bf = ml_dtypes.bfloat16
    tri = np.triu(np.ones((P, P), dtype=np.float32)).astype(bf)
    in_maps = []
    for c in range(N_CORES):
        b, g = c // 2, c % 2
        hs = slice(g * HL, (g + 1) * HL)
        m = {
            "xT": np.ascontiguousarray(x[b].T).astype(bf),
            "wq": np.ascontiguousarray(
                Wq[hs].transpose(1, 0, 2).reshape(C, HL * D)).astype(bf),
            "wk": np.ascontiguousarray(
                Wk[hs].transpose(1, 0, 2).reshape(C, HL * D)).astype(bf),
            "wv": np.ascontiguousarray(
                Wv[hs].transpose(1, 0, 2).reshape(C, HL * D)).astype(bf),
            "wps": np.ascontiguousarray(
                Wp[:, g * HL * D:(g + 1) * HL * D].T).astype(bf),
            "tri": tri,
            "onesb": np.ones((P, NS * 2), dtype=bf),
            "onesf": np.ones((P, D), dtype=np.float32),
        }
        in_maps.append(m)
    return in_maps


def assemble(results, bp):
    """Sum head-group partials per batch, add bias, transpose back."""
    out = np.empty((B, T, C), dtype=np.float32)
    for b in range(B):
        acc = results[2 * b]["o"] + results[2 * b + 1]["o"]  # [C, T]
        out[b] = acc.T + bp[None, :]
    return out


def kernel(x, Wq, Wk, Wv, Wp, bp):
    from concourse import bass_utils
    x = np.asarray(x, dtype=np.float32)
    nc = build_nc()
    in_maps = make_in_maps(np.asarray(x), np.asarray(Wq), np.asarray(Wk),
                           np.asarray(Wv), np.asarray(Wp))
    res = bass_utils.run_bass_kernel_spmd(nc, in_maps, core_ids=list(range(N_CORES)))
    return assemble(res.results, np.asarray(bp))
